# revision 2
# baseline (speedup 1.0000x reference)
"""Trainium2 Bass kernel: grouped MoE expert MLP (nn_ExpertGroup).

Strategy: expert parallelism across 8 NeuronCores. Tokens are sorted by
expert; core e runs expert e's two GEMMs:
    h = relu(x_e @ w_up[e].T) ** 2      (bf16, like the CUDA reference)
    y = h @ w_down[e].T
The host does the (free) token scatter/gather, the bf16 casts, and packs
every device-side DMA chunk into a fully contiguous DRAM block, so each
dma_start is 128 descriptors of 1-8KB (full bandwidth, ~0.3us descriptor
generation on the issuing queue) instead of 1024 strided 256B-1KB rows.

Start-latency layout (the old kernel's first real matmul waited until
14.5us for its operands): the first-needed chunks (x tokens 0:256 and
w_up j-tile 0) are triggered on the Scalar/Activation HWDGE queue, which
comes out of the NEFF preamble ~1.2us before the Sync queue. Remaining
w_up j-tiles stream from Sync one tile ahead of the PE's consumption.
GEMM1 runs in token chunks [256, 256, 512] so the first chunk's operands
are small; PE warm-up dummies bridge the preamble -> first-operand window
so the DVFS clock is at full speed when the real stream starts.

Device layout (per core, cap = padded local token count, default 1024):
    xT_sb  [128, 8192]       bf16  x_e.T packed per (chunk, d, tok)
    wuT_sb [128, 16, 8, 128] bf16  w_up[e].T packed per (j, d, col)
    wdT_sb [128, 16, 1024]   bf16  w_down[e].T packed per (j4, col)
    GEMM1: psum[j,t] = sum_d wuT[j,d].T @ xT[d,t]   (h in [H, T] layout)
    DVE:   relu -> bf16, square -> hsq SBUF [128, 16, cap]
    GEMM2: psum[t,i] = sum_j hsq[j,t].T @ wdT[j,i]  (y in [T, D] layout)
    DVE:   cast fp32 psum -> bf16 -> DMA to packed y [128, 8, 1024]

Built on bacc.Bacc (not raw Bass): Bacc.compile() legalizes semaphore
waits to the TRN2 limit of one wait per instruction (moving matmul waits
onto ldweights and splitting the rest into EventSemaphore instructions).
Raw Bass BIR fails walrus codegen with "Too many sync wait commands".
"""

import numpy as np
import ml_dtypes

import concourse.bass as bass
import concourse.mybir as mybir
import concourse.tile as tile
from concourse import bacc
from concourse.bass_utils import run_bass_kernel_spmd

T, D, H, E = 8192, 1024, 2048, 8
P = 128
N_CORES = 8
FD = 512           # GEMM2 matmul moving free dim (one PSUM bank of fp32)
C_CHUNKS = [(0, 256), (256, 256), (512, 512)]  # GEMM1 token chunks
N_WARM = 20        # PE warm-up dummy matmuls (bridge preamble->operands)
LAST_SPLIT = 256   # final GEMM2 group split size (drain shortening)


def _ensure_axon_ntff_hook():
    """The container's `antenv` stub lacks `axon_hooks`; if BASS_TRACE=1 is
    set, run_bass_kernel_spmd would crash importing it. Recreate the tiny
    registry and register the ctypes NTFF hook so tracing works (and never
    let this best-effort setup break the kernel)."""
    try:
        import antenv.axon_hooks  # noqa: F401
        return
    except ImportError:
        pass
    try:
        import sys
        import types

        import antenv
        from trn_agent_boot.trn_boot import _ntff_profile_via_ctypes

        mod = types.ModuleType("antenv.axon_hooks")
        mod._hook = _ntff_profile_via_ctypes("/opt/axon/libaxon_pjrt.so")
        mod.set_axon_ntff_profile_hook = lambda h: setattr(mod, "_hook", h)
        mod.get_axon_ntff_profile_hook = lambda: mod._hook
        sys.modules["antenv.axon_hooks"] = mod
        antenv.axon_hooks = mod
    except Exception:
        pass


_ensure_axon_ntff_hook()

_PROGRAM_CACHE: dict[int, "bass.Bass"] = {}
LAST_RESULT = None  # BassKernelResults of the most recent run (for harness use)


def _build_program(cap: int) -> "bass.Bass":
    n_d = D // P       # 8  contraction tiles of GEMM1
    n_j = H // P       # 16 H partition tiles
    n_t = cap // P     # token partition tiles (GEMM2 output)
    assert sum(l for _, l in C_CHUNKS) == cap
    bf16 = mybir.dt.bfloat16
    f32 = mybir.dt.float32

    nc = bacc.Bacc("TRN2", debug=False, num_devices=N_CORES)
    # Packed inputs: every tensor is consumed by exactly one dma_start and
    # is contiguous in DRAM in the order that DMA writes SBUF.
    xch = [
        nc.dram_tensor(f"x{c}", [P, n_d * l], bf16, kind="ExternalInput")
        for c, (_, l) in enumerate(C_CHUNKS)
    ]
    wuch = [
        nc.dram_tensor(f"wu{j}", [P, n_d * P], bf16, kind="ExternalInput")
        for j in range(n_j)
    ]
    wdch = [
        nc.dram_tensor(f"wd{c}", [P, 4 * D], bf16, kind="ExternalInput")
        for c in range(n_j // 4)
    ]
    y = nc.dram_tensor("y", [P, n_t * D], bf16, kind="ExternalOutput")

    with tile.TileContext(nc) as tc:
        with (
            tc.tile_pool(name="big", bufs=1) as big,
            tc.tile_pool(name="outp", bufs=4) as outp,
            tc.tile_pool(name="actp", bufs=4) as actp,
            tc.tile_pool(name="psum", bufs=7, space="PSUM") as psum,
            tc.tile_pool(name="warmp", bufs=1, space="PSUM") as warmp,
        ):
            xT_sb = big.tile([P, n_d * cap], bf16)
            wuT_sb = big.tile([P, n_j, n_d, P], bf16)
            wdT_sb = big.tile([P, n_j, D], bf16)
            hsq_sb = big.tile([P, n_j, cap], bf16)

            # PE warm-up: dummy matmuls with no DMA dependencies run while
            # the first input DMAs land. They keep the PE busy through the
            # HAM/DVFS activity window so the real matmul stream starts at
            # the full 2.4 GHz clock instead of the cold 1.2 GHz, and any
            # PE idle gap (which drops the clock again) is avoided.
            warm = big.tile([P, P], bf16)
            nc.gpsimd.memset(warm[:], 0.0)
            wps = warmp.tile([P, P], f32, tag="warm")
            for _ in range(N_WARM):
                nc.tensor.matmul(wps, warm[:], warm[:], start=True, stop=True)

            # --- input DMAs ---
            # Scalar/Activation HWDGE queue exits the preamble ~1.2us before
            # Sync, so it carries everything the first GEMM1 chunk needs
            # (x chunk 0 first: it is the larger transfer), then the rest of
            # x and all of w_down (needed only ~55us in, at GEMM2).
            nc.scalar.dma_start(
                out=xT_sb[:, 0:n_d * C_CHUNKS[0][1]], in_=xch[0][:]
            )
            nc.scalar.dma_start(out=wuT_sb[:, 0], in_=wuch[0][:])
            off = n_d * C_CHUNKS[0][1]
            for c in range(1, len(C_CHUNKS)):
                l = C_CHUNKS[c][1]
                nc.scalar.dma_start(
                    out=xT_sb[:, off:off + n_d * l], in_=xch[c][:]
                )
                off += n_d * l
            for c in range(n_j // 4):
                nc.scalar.dma_start(
                    out=wdT_sb[:, c * 4:(c + 1) * 4, :], in_=wdch[c][:]
                )
            # Sync queue: w_up j-tiles 1..15, one trigger each so each
            # tile's completion semaphore fires right behind the PE's
            # ~0.9us/j-tile consumption of the first token chunk.
            for j in range(1, n_j):
                nc.sync.dma_start(out=wuT_sb[:, j], in_=wuch[j][:])

            # --- GEMM1 + relu^2: hsq[j, t] ---
            for ci, (s, l) in enumerate(C_CHUNKS):
                xbase = n_d * s
                for j in range(n_j):
                    ps = psum.tile([P, FD], f32, tag="ps")
                    for d in range(n_d):
                        nc.tensor.matmul(
                            ps[:, 0:l],
                            wuT_sb[:, j, d],
                            xT_sb[:, xbase + d * l:xbase + (d + 1) * l],
                            start=(d == 0),
                            stop=(d == n_d - 1),
                        )
                    hr = actp.tile([P, FD], bf16, tag="hr")
                    nc.vector.tensor_relu(out=hr[:, 0:l], in_=ps[:, 0:l])
                    nc.vector.tensor_mul(
                        out=hsq_sb[:, j, s:s + l], in0=hr[:, 0:l], in1=hr[:, 0:l]
                    )

            # --- GEMM2: y[t, i] = sum_j hsq[j, t].T @ wdT[j, i] ---
            # The very last group is split so the final cast + output DMA
            # (the serial drain after the last matmul) moves half the data.
            groups = []
            for t in range(n_t):
                for ic in range(D // FD):
                    lo, hi = ic * FD, (ic + 1) * FD
                    if t == n_t - 1 and hi == D and LAST_SPLIT:
                        groups.append((t, lo, hi - LAST_SPLIT))
                        groups.append((t, hi - LAST_SPLIT, hi))
                    else:
                        groups.append((t, lo, hi))
            for t, lo, hi in groups:
                w = hi - lo
                ps = psum.tile([P, FD], f32, tag="ps")
                for j in range(n_j):
                    nc.tensor.matmul(
                        ps[:, 0:w],
                        hsq_sb[:, j, t * P:(t + 1) * P],
                        wdT_sb[:, j, lo:hi],
                        start=(j == 0),
                        stop=(j == n_j - 1),
                    )
                yt = outp.tile([P, FD], bf16, tag="yt")
                nc.vector.tensor_copy(out=yt[:, 0:w], in_=ps[:, 0:w])
                nc.sync.dma_start(
                    out=y[:, t * D + lo:t * D + hi], in_=yt[:, 0:w]
                )

    nc.compile()
    return nc


def _get_program(cap: int) -> "bass.Bass":
    nc = _PROGRAM_CACHE.get(cap)
    if nc is None:
        nc = _build_program(cap)
        _PROGRAM_CACHE[cap] = nc
    return nc


CAP = 1024  # tokens per core per round (the uniform T/E split = one round)


def _pack_pd(mat: np.ndarray, n_po: int) -> np.ndarray:
    """[n_po*128, W] row-major -> [128, n_po*W] with per-partition blocks
    ordered (po, col): the layout a [128, n_po, W] SBUF tile wants."""
    W = mat.shape[1]
    return np.ascontiguousarray(
        mat.reshape(n_po, P, W).transpose(1, 0, 2).reshape(P, n_po * W)
    )


def kernel(x, num_tokens_per_expert, w_up, w_down, _trace=False):
    global LAST_RESULT
    bf = ml_dtypes.bfloat16
    x = np.asarray(x)
    counts = np.asarray(num_tokens_per_expert).astype(np.int64)
    w_up = np.asarray(w_up)
    w_down = np.asarray(w_down)
    n_tok = x.shape[0]
    assert counts.shape == (E,) and int(counts.sum()) == n_tok
    offsets = np.zeros(E, dtype=np.int64)
    offsets[1:] = np.cumsum(counts)[:-1]

    nc = _get_program(CAP)
    n_d, n_j, n_t = D // P, H // P, CAP // P

    # Work list: split each expert's contiguous token segment into slots of
    # <= CAP tokens; process 8 slots per SPMD round. The uniform T/E = 1024
    # split is exactly one round of 8 slots.
    slots = []
    for e in range(E):
        cnt, off = int(counts[e]), int(offsets[e])
        for s in range(0, cnt, CAP):
            slots.append((e, off + s, min(CAP, cnt - s)))

    weight_cache = {}

    def expert_weights(e):
        if e not in weight_cache:
            wuT = np.ascontiguousarray(w_up[e].astype(bf).T)    # [D, H]
            wdT = np.ascontiguousarray(w_down[e].astype(bf).T)  # [H, D]
            wu3 = wuT.reshape(n_d, P, H)
            m = {
                f"wu{j}": np.ascontiguousarray(
                    wu3[:, :, j * P:(j + 1) * P].transpose(1, 0, 2)
                ).reshape(P, n_d * P)
                for j in range(n_j)
            }
            wd3 = wdT.reshape(n_j, P, D)
            for c in range(n_j // 4):
                m[f"wd{c}"] = np.ascontiguousarray(
                    wd3[c * 4:(c + 1) * 4].transpose(1, 0, 2)
                ).reshape(P, 4 * D)
            weight_cache[e] = m
        return weight_cache[e]

    out = np.zeros((n_tok, D), dtype=x.dtype)
    zero_map = None
    for r0 in range(0, len(slots), N_CORES):
        round_slots = slots[r0:r0 + N_CORES]
        in_maps = []
        for e, off, cnt in round_slots:
            xs = np.zeros((CAP, D), dtype=bf)
            xs[:cnt] = x[off:off + cnt].astype(bf)
            xT = np.ascontiguousarray(xs.T)  # [D, CAP]
            xT3 = xT.reshape(n_d, P, CAP)
            im = dict(expert_weights(e))
            for c, (s, l) in enumerate(C_CHUNKS):
                im[f"x{c}"] = np.ascontiguousarray(
                    xT3[:, :, s:s + l].transpose(1, 0, 2)
                ).reshape(P, n_d * l)
            in_maps.append(im)
        while len(in_maps) < N_CORES:  # idle cores in the last round
            if zero_map is None:
                zero_map = {
                    f"x{c}": np.zeros((P, n_d * l), dtype=bf)
                    for c, (_, l) in enumerate(C_CHUNKS)
                }
                zero_map.update({
                    f"wu{j}": np.zeros((P, n_d * P), dtype=bf)
                    for j in range(n_j)
                })
                zero_map.update({
                    f"wd{c}": np.zeros((P, 4 * D), dtype=bf)
                    for c in range(n_j // 4)
                })
            in_maps.append(zero_map)

        res = run_bass_kernel_spmd(
            nc, in_maps, core_ids=list(range(N_CORES)), trace=_trace
        )
        LAST_RESULT = res
        for i, (e, off, cnt) in enumerate(round_slots):
            yp = res.results[i]["y"].reshape(P, n_t, D).transpose(1, 0, 2)
            out[off:off + cnt] = yp.reshape(CAP, D)[:cnt].astype(x.dtype)
    return out


# revision 3
# speedup vs baseline: 1.2560x; 1.2560x over previous
"""Trainium2 Bass kernel: grouped MoE expert MLP (nn_ExpertGroup).

Strategy: expert parallelism across 8 NeuronCores. Tokens are sorted by
expert; core e runs expert e's two GEMMs:
    h = relu(x_e @ w_up[e].T) ** 2      (bf16, like the CUDA reference)
    y = h @ w_down[e].T
The host does the (free) token scatter/gather, the bf16 casts, and the
weight transposes so every device-side DMA is contiguous.

Device layout (per core, cap = padded local token count, default 1024):
    xT  (D=1024, cap)  bf16   x_e.T               -> SBUF [128, 8, cap]
    wuT (D=1024, H=2048) bf16 w_up[e].T           -> SBUF [128, 8, 2048]
    wdT (H=2048, D=1024) bf16 w_down[e].T         -> SBUF [128, 16, 1024]
    GEMM1: psum[j,t] = sum_d wuT[d,j].T @ xT[d,t]   (h in [H, T] layout)
    DVE:   relu -> bf16, square -> hsq SBUF [128, 16, cap]
    GEMM2: psum[t,i] = sum_j hsq[j,t].T @ wdT[j,i]  (y in [T, D] layout)
    DVE:   cast fp32 psum -> bf16 y -> DMA out

Built on bacc.Bacc (not raw Bass): Bacc.compile() legalizes semaphore
waits to the TRN2 limit of one wait per instruction (moving matmul waits
onto ldweights and splitting the rest into EventSemaphore instructions).
Raw Bass BIR fails walrus codegen with "Too many sync wait commands".
"""

import numpy as np
import ml_dtypes

import concourse.bass as bass
import concourse.mybir as mybir
import concourse.tile as tile
from concourse import bacc
from concourse.bass_utils import run_bass_kernel_spmd

T, D, H, E = 8192, 1024, 2048, 8
P = 128
N_CORES = 8
FD = 512  # matmul moving free dim (one PSUM bank of fp32)


def _ensure_axon_ntff_hook():
    """The container's `antenv` stub lacks `axon_hooks`; if BASS_TRACE=1 is
    set, run_bass_kernel_spmd would crash importing it. Recreate the tiny
    registry and register the ctypes NTFF hook so tracing works (and never
    let this best-effort setup break the kernel)."""
    try:
        import antenv.axon_hooks  # noqa: F401
        return
    except ImportError:
        pass
    try:
        import sys
        import types

        import antenv
        from trn_agent_boot.trn_boot import _ntff_profile_via_ctypes

        mod = types.ModuleType("antenv.axon_hooks")
        mod._hook = _ntff_profile_via_ctypes("/opt/axon/libaxon_pjrt.so")
        mod.set_axon_ntff_profile_hook = lambda h: setattr(mod, "_hook", h)
        mod.get_axon_ntff_profile_hook = lambda: mod._hook
        sys.modules["antenv.axon_hooks"] = mod
        antenv.axon_hooks = mod
    except Exception:
        pass


_ensure_axon_ntff_hook()

_PROGRAM_CACHE: dict[int, "bass.Bass"] = {}
LAST_RESULT = None  # BassKernelResults of the most recent run (for harness use)


def _build_program(cap: int) -> "bass.Bass":
    assert cap % FD == 0
    n_d = D // P       # 8  contraction tiles of GEMM1
    n_j = H // P       # 16 H partition tiles
    n_tc = cap // FD   # token chunks (moving operand of GEMM1)
    n_t = cap // P     # token partition tiles (GEMM2 output)
    bf16 = mybir.dt.bfloat16
    f32 = mybir.dt.float32

    nc = bacc.Bacc("TRN2", debug=False, num_devices=N_CORES)
    xT = nc.dram_tensor("xT", [D, cap], bf16, kind="ExternalInput")
    wuT = nc.dram_tensor("wuT", [D, H], bf16, kind="ExternalInput")
    wdT = nc.dram_tensor("wdT", [H, D], bf16, kind="ExternalInput")
    y = nc.dram_tensor("y", [cap, D], bf16, kind="ExternalOutput")

    xT3 = xT[:].rearrange("(po pi) f -> pi po f", pi=P)    # [128, 8, cap]
    wuT3 = wuT[:].rearrange("(po pi) f -> pi po f", pi=P)  # [128, 8, 2048]
    wdT3 = wdT[:].rearrange("(po pi) f -> pi po f", pi=P)  # [128, 16, 1024]
    y3 = y[:].rearrange("(po pi) f -> pi po f", pi=P)      # [128, n_t, 1024]

    with tile.TileContext(nc) as tc:
        with (
            tc.tile_pool(name="big", bufs=1) as big,
            tc.tile_pool(name="outp", bufs=4) as outp,
            tc.tile_pool(name="actp", bufs=4) as actp,
            tc.tile_pool(name="psum", bufs=7, space="PSUM") as psum,
            tc.tile_pool(name="warmp", bufs=1, space="PSUM") as warmp,
        ):
            xT_sb = big.tile([P, n_d, cap], bf16)
            wuT_sb = big.tile([P, n_d, H], bf16)
            wdT_sb = big.tile([P, n_j, D], bf16)
            hsq_sb = big.tile([P, n_j, cap], bf16)

            # PE warm-up: ~80 dummy matmuls with no DMA dependencies run
            # while the input DMAs stream in. They keep the PE busy through
            # the HAM activity window so the real matmul stream starts at
            # the full 2.4 GHz clock instead of the cold 1.2 GHz.
            warm = big.tile([P, P], bf16)
            nc.gpsimd.memset(warm[:], 0.0)
            wps = warmp.tile([P, P], f32, tag="warm")
            # 60 dummies end ~12.5us; the first (half-width) real group's
            # operands (0.75 MB) are ready ~11.4us, so the real stream
            # starts straight off the warmup with HAM already at 8/8.
            for _ in range(60):
                nc.tensor.matmul(wps, warm[:], warm[:], start=True, stop=True)

            # Input DMAs, ordered so the first GEMM1 group's operands land
            # first (wuT columns for j=0, then the first token chunk). One
            # dma_start already spreads across all 16 SDMA engines, and the
            # HWDGE ring is FIFO, so issue order = arrival order.
            # wuT arrives as 16 per-j-tile chunks: a chunk's completion
            # semaphore fires ~1-2us after its data lands, so fine chunks
            # keep every sem comfortably ahead of the PE's ~1.7us/j-tile
            # consumption; one big chunk would stall the next j group.
            nc.sync.dma_start(out=wuT_sb[:, :, 0:P], in_=wuT3[:, :, 0:P])
            nc.sync.dma_start(
                out=xT_sb[:, :, 0:FD], in_=xT3[:, :, 0:FD]
            )
            for j in range(1, n_j):
                nc.sync.dma_start(
                    out=wuT_sb[:, :, j * P:(j + 1) * P],
                    in_=wuT3[:, :, j * P:(j + 1) * P],
                )
            for c in range(1, n_tc):
                nc.sync.dma_start(
                    out=xT_sb[:, :, c * FD:(c + 1) * FD],
                    in_=xT3[:, :, c * FD:(c + 1) * FD],
                )
            for c in range(n_j // 4):
                nc.sync.dma_start(
                    out=wdT_sb[:, c * 4:(c + 1) * 4, :],
                    in_=wdT3[:, c * 4:(c + 1) * 4, :],
                )

            # GEMM1 + relu^2: hsq[j, t] (token chunk outer so GEMM2 of the
            # first half can start while the second half computes)
            for c in range(n_tc):
                for j in range(n_j):
                    ps = psum.tile([P, FD], f32, tag="ps")
                    for d in range(n_d):
                        nc.tensor.matmul(
                            ps,
                            wuT_sb[:, d, j * P:(j + 1) * P],
                            xT_sb[:, d, c * FD:(c + 1) * FD],
                            start=(d == 0),
                            stop=(d == n_d - 1),
                        )
                    hr = actp.tile([P, FD], bf16, tag="hr")
                    nc.vector.tensor_relu(out=hr, in_=ps)
                    nc.vector.tensor_mul(
                        out=hsq_sb[:, j, c * FD:(c + 1) * FD], in0=hr, in1=hr
                    )

            # GEMM2: y[t, i] = sum_j hsq[j, t].T @ wdT[j, i]
            for t in range(n_t):
                for ic in range(D // FD):
                    ps = psum.tile([P, FD], f32, tag="ps")
                    for j in range(n_j):
                        nc.tensor.matmul(
                            ps,
                            hsq_sb[:, j, t * P:(t + 1) * P],
                            wdT_sb[:, j, ic * FD:(ic + 1) * FD],
                            start=(j == 0),
                            stop=(j == n_j - 1),
                        )
                    yt = outp.tile([P, FD], bf16, tag="yt")
                    nc.vector.tensor_copy(out=yt, in_=ps)
                    nc.sync.dma_start(
                        out=y3[:, t, ic * FD:(ic + 1) * FD], in_=yt
                    )

    nc.compile()
    return nc


def _get_program(cap: int) -> "bass.Bass":
    nc = _PROGRAM_CACHE.get(cap)
    if nc is None:
        nc = _build_program(cap)
        _PROGRAM_CACHE[cap] = nc
    return nc


CAP = 1024  # tokens per core per round (the uniform T/E split = one round)


def kernel(x, num_tokens_per_expert, w_up, w_down, _trace=False):
    global LAST_RESULT
    bf = ml_dtypes.bfloat16
    x = np.asarray(x)
    counts = np.asarray(num_tokens_per_expert).astype(np.int64)
    w_up = np.asarray(w_up)
    w_down = np.asarray(w_down)
    n_tok = x.shape[0]
    assert counts.shape == (E,) and int(counts.sum()) == n_tok
    offsets = np.zeros(E, dtype=np.int64)
    offsets[1:] = np.cumsum(counts)[:-1]

    nc = _get_program(CAP)

    # Work list: split each expert's contiguous token segment into slots of
    # <= CAP tokens; process 8 slots per SPMD round. The uniform T/E = 1024
    # split is exactly one round of 8 slots.
    slots = []
    for e in range(E):
        cnt, off = int(counts[e]), int(offsets[e])
        for s in range(0, cnt, CAP):
            slots.append((e, off + s, min(CAP, cnt - s)))

    wuT_cache = {}
    wdT_cache = {}

    def expert_weights(e):
        if e not in wuT_cache:
            wuT_cache[e] = np.ascontiguousarray(w_up[e].astype(bf).T)
            wdT_cache[e] = np.ascontiguousarray(w_down[e].astype(bf).T)
        return wuT_cache[e], wdT_cache[e]

    out = np.zeros((n_tok, D), dtype=x.dtype)
    zero_map = None
    for r0 in range(0, len(slots), N_CORES):
        round_slots = slots[r0:r0 + N_CORES]
        in_maps = []
        for e, off, cnt in round_slots:
            xs = np.zeros((CAP, D), dtype=bf)
            xs[:cnt] = x[off:off + cnt].astype(bf)
            wuT, wdT = expert_weights(e)
            in_maps.append({
                "xT": np.ascontiguousarray(xs.T), "wuT": wuT, "wdT": wdT,
            })
        while len(in_maps) < N_CORES:  # idle cores in the last round
            if zero_map is None:
                zero_map = {
                    "xT": np.zeros((D, CAP), dtype=bf),
                    "wuT": np.zeros((D, H), dtype=bf),
                    "wdT": np.zeros((H, D), dtype=bf),
                }
            in_maps.append(zero_map)

        res = run_bass_kernel_spmd(
            nc, in_maps, core_ids=list(range(N_CORES)), trace=_trace
        )
        LAST_RESULT = res
        for i, (e, off, cnt) in enumerate(round_slots):
            out[off:off + cnt] = res.results[i]["y"][:cnt].astype(x.dtype)
    return out


# revision 4
# speedup vs baseline: 1.2620x; 1.0048x over previous
"""Trainium2 Bass kernel: grouped MoE expert MLP (nn_ExpertGroup).

Strategy: expert parallelism across 8 NeuronCores. Tokens are sorted by
expert; core e runs expert e's two GEMMs:
    h = relu(x_e @ w_up[e].T) ** 2      (bf16, like the CUDA reference)
    y = h @ w_down[e].T
The host does the (free) token scatter/gather, the bf16 casts, and packs
every device-side DMA chunk into a fully contiguous DRAM block, so each
dma_start is 128 descriptors of 1-8KB at full transfer rate instead of
1024 strided 256B-1KB rows.

Timing-critical discipline (measured on hardware): the PE's DVFS boost
clock (2.4 GHz vs 2.0 GHz base) is earned during an early activity
window and is forfeited FOR THE WHOLE RUN if the PE idles more than
~2us. So (1) warm-up dummies bridge the preamble to the first operand
arrival with no gap, and (2) the input DMA schedule must keep every
w_up j-tile's completion semaphore ahead of the PE's consumption. Input
DMAs are split across the two HWDGE queues (Sync + Scalar/Activation,
each ~150-230 GB/s, ~350 GB/s aggregate) so the first GEMM1 chunk's
operands (x tokens 0:256 + w_up j0) land ~2us earlier than a single
queue could, and the j-tile stream is supplied from both queues
alternately at ~0.75us cadence vs the PE's 0.88us/j-tile demand.

Device layout (per core, cap = padded local token count, default 1024):
    xT_sb  [128, 8*cap]      bf16  x_e.T packed per (chunk, d, tok)
    wuT_sb [128, 16, 8, 128] bf16  w_up[e].T packed per (j, d, col)
    wdT_sb [128, 16, 1024]   bf16  w_down[e].T packed per (j4, col)
    GEMM1: psum[j,t] = sum_d wuT[j,d].T @ xT[d,t]   (h in [H, T] layout)
           token chunks [256, 256, 512] so the first chunk needs only
           512KB of x before the stream can start
    DVE:   relu -> bf16, square -> hsq SBUF [128, 16, cap]
    GEMM2: psum[t,i] = sum_j hsq[j,t].T @ wdT[j,i]  (y in [T, D] layout)
    DVE:   cast fp32 psum -> bf16 -> DMA to packed y [128, 8, 1024]
    The final GEMM2 group is split 256+256 so the serial drain after the
    last matmul (cast + output DMA + completion) moves half the bytes.

Built on bacc.Bacc (not raw Bass): Bacc.compile() legalizes semaphore
waits to the TRN2 limit of one wait per instruction (moving matmul waits
onto ldweights and splitting the rest into EventSemaphore instructions).
Raw Bass BIR fails walrus codegen with "Too many sync wait commands".
"""

import numpy as np
import ml_dtypes

import concourse.bass as bass
import concourse.mybir as mybir
import concourse.tile as tile
from concourse import bacc
from concourse.bass_utils import run_bass_kernel_spmd

T, D, H, E = 8192, 1024, 2048, 8
P = 128
N_CORES = 8
FD = 512           # GEMM2 matmul moving free dim (one PSUM bank of fp32)
C_CHUNKS = [(0, 256), (256, 256), (512, 512)]  # GEMM1 token chunks
N_WARM = 66        # PE warm-up dummies (bridge preamble -> first operands)
LAST_SPLIT = 256   # final GEMM2 group split size (drain shortening)


def _ensure_axon_ntff_hook():
    """The container's `antenv` stub lacks `axon_hooks`; if BASS_TRACE=1 is
    set, run_bass_kernel_spmd would crash importing it. Recreate the tiny
    registry and register the ctypes NTFF hook so tracing works (and never
    let this best-effort setup break the kernel)."""
    try:
        import antenv.axon_hooks  # noqa: F401
        return
    except ImportError:
        pass
    try:
        import sys
        import types

        import antenv
        from trn_agent_boot.trn_boot import _ntff_profile_via_ctypes

        mod = types.ModuleType("antenv.axon_hooks")
        mod._hook = _ntff_profile_via_ctypes("/opt/axon/libaxon_pjrt.so")
        mod.set_axon_ntff_profile_hook = lambda h: setattr(mod, "_hook", h)
        mod.get_axon_ntff_profile_hook = lambda: mod._hook
        sys.modules["antenv.axon_hooks"] = mod
        antenv.axon_hooks = mod
    except Exception:
        pass


_ensure_axon_ntff_hook()

_PROGRAM_CACHE: dict[int, "bass.Bass"] = {}
LAST_RESULT = None  # BassKernelResults of the most recent run (for harness use)


def _build_program(cap: int) -> "bass.Bass":
    n_d = D // P       # 8  contraction tiles of GEMM1
    n_j = H // P       # 16 H partition tiles
    n_t = cap // P     # token partition tiles (GEMM2 output)
    assert sum(l for _, l in C_CHUNKS) == cap
    bf16 = mybir.dt.bfloat16
    f32 = mybir.dt.float32

    nc = bacc.Bacc("TRN2", debug=False, num_devices=N_CORES)
    # Packed inputs: every tensor is consumed by exactly one dma_start and
    # is contiguous in DRAM in the order that DMA writes SBUF.
    xch = [
        nc.dram_tensor(f"x{c}", [P, n_d * l], bf16, kind="ExternalInput")
        for c, (_, l) in enumerate(C_CHUNKS)
    ]
    wuch = [
        nc.dram_tensor(f"wu{j}", [P, n_d * P], bf16, kind="ExternalInput")
        for j in range(n_j)
    ]
    wdch = [
        nc.dram_tensor(f"wd{c}", [P, 4 * D], bf16, kind="ExternalInput")
        for c in range(n_j // 4)
    ]
    y = nc.dram_tensor("y", [P, n_t * D], bf16, kind="ExternalOutput")

    with tile.TileContext(nc) as tc:
        with (
            tc.tile_pool(name="big", bufs=1) as big,
            tc.tile_pool(name="outp", bufs=4) as outp,
            tc.tile_pool(name="actp", bufs=4) as actp,
            tc.tile_pool(name="psum", bufs=7, space="PSUM") as psum,
            tc.tile_pool(name="warmp", bufs=1, space="PSUM") as warmp,
        ):
            xT_sb = big.tile([P, n_d * cap], bf16)
            wuT_sb = big.tile([P, n_j, n_d, P], bf16)
            wdT_sb = big.tile([P, n_j, D], bf16)
            hsq_sb = big.tile([P, n_j, cap], bf16)

            # PE warm-up: dummy matmuls with no DMA dependencies run while
            # the first input DMAs land, ending right at the measured
            # first-operand semaphore time. Any >2us PE idle here loses the
            # DVFS boost clock for the whole run (-20% on every matmul).
            warm = big.tile([P, P], bf16)
            nc.gpsimd.memset(warm[:], 0.0)
            wps = warmp.tile([P, P], f32, tag="warm")
            for _ in range(N_WARM):
                nc.tensor.matmul(wps, warm[:], warm[:], start=True, stop=True)

            # --- input DMAs, split across the two HWDGE queues ---
            # Transfers on one queue are processed in issue order at
            # ~150-230 GB/s, so each queue is an independent supply lane.
            # Scalar lane: x chunk 0 first (the start-gating transfer),
            # then even w_up j-tiles, then x chunks 1-2, then w_down tail.
            # Sync lane: w_up j0 (the other start-gating transfer), then
            # odd w_up j-tiles, then the first two w_down chunks.
            q_scalar = [("x", 0)] + [("wu", j) for j in range(2, n_j, 2)]
            q_scalar += [("x", 1), ("x", 2), ("wd", 2), ("wd", 3)]
            q_sync = [("wu", 0)] + [("wu", j) for j in range(1, n_j, 2)]
            q_sync += [("wd", 0), ("wd", 1)]

            def issue(eng, kind, i):
                if kind == "x":
                    s, l = C_CHUNKS[i]
                    eng.dma_start(
                        out=xT_sb[:, n_d * s:n_d * (s + l)], in_=xch[i][:]
                    )
                elif kind == "wu":
                    eng.dma_start(out=wuT_sb[:, i], in_=wuch[i][:])
                else:
                    eng.dma_start(
                        out=wdT_sb[:, i * 4:(i + 1) * 4, :], in_=wdch[i][:]
                    )

            for kind, i in q_scalar:
                issue(nc.scalar, kind, i)
            for kind, i in q_sync:
                issue(nc.sync, kind, i)

            # --- GEMM1 + relu^2: hsq[j, t] ---
            for ci, (s, l) in enumerate(C_CHUNKS):
                xbase = n_d * s
                for j in range(n_j):
                    ps = psum.tile([P, FD], f32, tag="ps")
                    for d in range(n_d):
                        nc.tensor.matmul(
                            ps[:, 0:l],
                            wuT_sb[:, j, d],
                            xT_sb[:, xbase + d * l:xbase + (d + 1) * l],
                            start=(d == 0),
                            stop=(d == n_d - 1),
                        )
                    hr = actp.tile([P, FD], bf16, tag="hr")
                    nc.vector.tensor_relu(out=hr[:, 0:l], in_=ps[:, 0:l])
                    nc.vector.tensor_mul(
                        out=hsq_sb[:, j, s:s + l], in0=hr[:, 0:l], in1=hr[:, 0:l]
                    )

            # --- GEMM2: y[t, i] = sum_j hsq[j, t].T @ wdT[j, i] ---
            groups = []
            for t in range(n_t):
                for ic in range(D // FD):
                    lo, hi = ic * FD, (ic + 1) * FD
                    if t == n_t - 1 and hi == D and LAST_SPLIT:
                        groups.append((t, lo, hi - LAST_SPLIT))
                        groups.append((t, hi - LAST_SPLIT, hi))
                    else:
                        groups.append((t, lo, hi))
            for t, lo, hi in groups:
                w = hi - lo
                ps = psum.tile([P, FD], f32, tag="ps")
                for j in range(n_j):
                    nc.tensor.matmul(
                        ps[:, 0:w],
                        hsq_sb[:, j, t * P:(t + 1) * P],
                        wdT_sb[:, j, lo:hi],
                        start=(j == 0),
                        stop=(j == n_j - 1),
                    )
                yt = outp.tile([P, FD], bf16, tag="yt")
                nc.vector.tensor_copy(out=yt[:, 0:w], in_=ps[:, 0:w])
                nc.sync.dma_start(
                    out=y[:, t * D + lo:t * D + hi], in_=yt[:, 0:w]
                )

    nc.compile()
    return nc


def _get_program(cap: int) -> "bass.Bass":
    nc = _PROGRAM_CACHE.get(cap)
    if nc is None:
        nc = _build_program(cap)
        _PROGRAM_CACHE[cap] = nc
    return nc


CAP = 1024  # tokens per core per round (the uniform T/E split = one round)


def kernel(x, num_tokens_per_expert, w_up, w_down, _trace=False):
    global LAST_RESULT
    bf = ml_dtypes.bfloat16
    x = np.asarray(x)
    counts = np.asarray(num_tokens_per_expert).astype(np.int64)
    w_up = np.asarray(w_up)
    w_down = np.asarray(w_down)
    n_tok = x.shape[0]
    assert counts.shape == (E,) and int(counts.sum()) == n_tok
    offsets = np.zeros(E, dtype=np.int64)
    offsets[1:] = np.cumsum(counts)[:-1]

    nc = _get_program(CAP)
    n_d, n_j, n_t = D // P, H // P, CAP // P

    # Work list: split each expert's contiguous token segment into slots of
    # <= CAP tokens; process 8 slots per SPMD round. The uniform T/E = 1024
    # split is exactly one round of 8 slots.
    slots = []
    for e in range(E):
        cnt, off = int(counts[e]), int(offsets[e])
        for s in range(0, cnt, CAP):
            slots.append((e, off + s, min(CAP, cnt - s)))

    weight_cache = {}

    def expert_weights(e):
        if e not in weight_cache:
            wuT = np.ascontiguousarray(w_up[e].astype(bf).T)    # [D, H]
            wdT = np.ascontiguousarray(w_down[e].astype(bf).T)  # [H, D]
            wu3 = wuT.reshape(n_d, P, H)
            m = {
                f"wu{j}": np.ascontiguousarray(
                    wu3[:, :, j * P:(j + 1) * P].transpose(1, 0, 2)
                ).reshape(P, n_d * P)
                for j in range(n_j)
            }
            wd3 = wdT.reshape(n_j, P, D)
            for c in range(n_j // 4):
                m[f"wd{c}"] = np.ascontiguousarray(
                    wd3[c * 4:(c + 1) * 4].transpose(1, 0, 2)
                ).reshape(P, 4 * D)
            weight_cache[e] = m
        return weight_cache[e]

    out = np.zeros((n_tok, D), dtype=x.dtype)
    zero_map = None
    for r0 in range(0, len(slots), N_CORES):
        round_slots = slots[r0:r0 + N_CORES]
        in_maps = []
        for e, off, cnt in round_slots:
            xs = np.zeros((CAP, D), dtype=bf)
            xs[:cnt] = x[off:off + cnt].astype(bf)
            xT = np.ascontiguousarray(xs.T)  # [D, CAP]
            xT3 = xT.reshape(n_d, P, CAP)
            im = dict(expert_weights(e))
            for c, (s, l) in enumerate(C_CHUNKS):
                im[f"x{c}"] = np.ascontiguousarray(
                    xT3[:, :, s:s + l].transpose(1, 0, 2)
                ).reshape(P, n_d * l)
            in_maps.append(im)
        while len(in_maps) < N_CORES:  # idle cores in the last round
            if zero_map is None:
                zero_map = {
                    f"x{c}": np.zeros((P, n_d * l), dtype=bf)
                    for c, (_, l) in enumerate(C_CHUNKS)
                }
                zero_map.update({
                    f"wu{j}": np.zeros((P, n_d * P), dtype=bf)
                    for j in range(n_j)
                })
                zero_map.update({
                    f"wd{c}": np.zeros((P, 4 * D), dtype=bf)
                    for c in range(n_j // 4)
                })
            in_maps.append(zero_map)

        res = run_bass_kernel_spmd(
            nc, in_maps, core_ids=list(range(N_CORES)), trace=_trace
        )
        LAST_RESULT = res
        for i, (e, off, cnt) in enumerate(round_slots):
            yp = res.results[i]["y"].reshape(P, n_t, D).transpose(1, 0, 2)
            out[off:off + cnt] = yp.reshape(CAP, D)[:cnt].astype(x.dtype)
    return out
